# revision 20
# baseline (speedup 1.0000x reference)
"""Trainium2 Bass kernel for a biased transformer encoder layer.

Full (unsharded) inputs -> full output. Internally shards data-parallel over
batch B across 8 NeuronCores (one batch element per core). The bias tensor
(256MB) dominates memory traffic; it is host-exponentiated/transposed to bf16
so attention works in "transposed scores" layout and softmax is
exp(qk)*exp(bias) with a matmul-computed (pre-broadcast) denominator.

v2: software-pipelined attention (PE issues next group's score matmuls before
the previous group's ctx matmuls so it never idles on the exp->mult chain),
all bias multiplies on DVE as single [128,2048] bf16 tensor_tensor ops,
layernorm rsqrt via batched ln/exp on ACT (single activation table set for
the whole kernel), packed single-DMA weight/src loads.
"""

import numpy as np
import ml_dtypes

import concourse.mybir as mybir
import concourse.tile as tile
from concourse import bacc
from concourse.bass_utils import run_bass_kernel_spmd

# ---- problem constants (hardcoded per contract) ----
S = 1024
B = 8
D = 256
H = 8
HD = D // H          # 32
DFF = 1024
EPS = 1e-5
N_CORES = 8
NT = S // 128        # 8 s-tiles / t-tiles

F32 = mybir.dt.float32
BF16 = mybir.dt.bfloat16
bf16 = ml_dtypes.bfloat16

_CACHE = {}

# packed bf16 weight column layout
_WCOLS = {}
_off = 0
for _name, _w in [("identb", 128), ("wqkT0", 512), ("wqkT1", 512),
                  ("wvxT0", 512), ("wvxT1", 512), ("woT0", 256), ("woT1", 256),
                  ("w1T0", 1024), ("w1T1", 1024)] + [(f"w2T{k}", 256) for k in range(8)]:
    _WCOLS[_name] = (_off, _off + _w)
    _off += _w
_NWC = _off  # 6784


def _install_axon_hooks_shim():
    """Make `trace=True` degrade gracefully if antenv.axon_hooks is missing."""
    import sys, types
    try:
        import antenv  # noqa
    except ImportError:
        return
    if "antenv.axon_hooks" in sys.modules:
        return
    try:
        import antenv.axon_hooks  # noqa
    except ImportError:
        import antenv
        mod = types.ModuleType("antenv.axon_hooks")
        _hook = [None]
        mod.set_axon_ntff_profile_hook = lambda h: _hook.__setitem__(0, h)
        mod.get_axon_ntff_profile_hook = lambda: _hook[0]
        sys.modules["antenv.axon_hooks"] = mod
        antenv.axon_hooks = mod


def _patched_act_tables(orig_fn):
    """Return a get_activation_tables wrapper that strips Exp/Ln from every
    set except natural_log_exp_and_others, so the table-load pass resolves
    both functions to the one set that holds them jointly (a single
    ACT_TABLE_LOAD for the whole kernel instead of thrashing between the
    exp-only and ln-only sets)."""
    def patched(arch):
        tabs = {k: set(v) for k, v in orig_fn(arch).items()}
        combined = "natural_log_exp_and_others"
        if combined in tabs:
            EXP = mybir.ActivationFunctionType.Exp
            LNF = mybir.ActivationFunctionType.Ln
            if EXP in tabs[combined] and LNF in tabs[combined]:
                for name, fns in tabs.items():
                    if name != combined:
                        fns.discard(EXP)
                        fns.discard(LNF)
        return tabs
    return patched


def _build(flags):
    """Build the Bass program (shared by all 8 cores, SPMD)."""
    (aff1, aff2, has_bqk, has_bo, has_b1, has_b2) = flags
    orig_tables = bacc.get_activation_tables
    bacc.get_activation_tables = _patched_act_tables(orig_tables)
    try:
        return _build_inner(flags)
    finally:
        bacc.get_activation_tables = orig_tables


def _build_inner(flags):
    (aff1, aff2, has_bqk, has_bo, has_b1, has_b2) = flags
    nc = bacc.Bacc("TRN2", debug=False, num_devices=N_CORES, enable_asserts=True)

    # ---- DRAM tensors (per-core inputs) ----
    src_d = nc.dram_tensor("src", [S, D], F32, kind="ExternalInput")
    wpack_d = nc.dram_tensor("wpack", [128, _NWC], BF16, kind="ExternalInput")
    bvx_d = nc.dram_tensor("bvx", [128, 2 * D], F32, kind="ExternalInput")
    ebg_d = nc.dram_tensor("ebg", [32, 128, 2048], BF16, kind="ExternalInput")
    if has_bqk:
        bqk_d = nc.dram_tensor("bqk", [128, 4], F32, kind="ExternalInput")
    if aff1:
        g1b_d = nc.dram_tensor("g1b", [128, D], F32, kind="ExternalInput")
        be1b_d = nc.dram_tensor("be1b", [128, D], F32, kind="ExternalInput")
    if aff2:
        g2b_d = nc.dram_tensor("g2b", [128, D], F32, kind="ExternalInput")
        be2b_d = nc.dram_tensor("be2b", [128, D], F32, kind="ExternalInput")
    if has_bo:
        bob_d = nc.dram_tensor("bob", [128, D], F32, kind="ExternalInput")
    if has_b1:
        b1c_d = nc.dram_tensor("b1c", [128, DFF // 128], F32, kind="ExternalInput")
    if has_b2:
        b2c_d = nc.dram_tensor("b2c", [128, D // 128], F32, kind="ExternalInput")
    out_d = nc.dram_tensor("out", [S, D], F32, kind="ExternalOutput")
    import os as _os
    _DBG = bool(_os.environ.get("KERNEL_DEBUG"))
    if _DBG:
        dbg_rs1 = nc.dram_tensor("dbg_rs1", [128, NT], F32, kind="ExternalOutput")
        dbg_rr = nc.dram_tensor("dbg_rr", [32, S], F32, kind="ExternalOutput")
        dbg_ctxT = nc.dram_tensor("dbg_ctxT", [128, S], F32, kind="ExternalOutput")
        dbg_den = nc.dram_tensor("dbg_den", [32, S], F32, kind="ExternalOutput")

    LN = mybir.ActivationFunctionType.Ln
    EXP = mybir.ActivationFunctionType.Exp
    RELU = mybir.ActivationFunctionType.Relu

    with tile.TileContext(nc, trace_sim=True) as tc:
        with tc.tile_pool(name="persist", bufs=1) as pp:
            # ---- packed loads ----
            eps_t = pp.tile([128, 1], F32, tag="eps_t", name="eps_t")
            nc.gpsimd.memset(eps_t[:], EPS)
            # prime the ln/exp activation table while DMAs are in flight
            prime = pp.tile([128, 1], F32, tag="prime", name="prime")
            nc.scalar.activation(prime[:], eps_t[:], mybir.ActivationFunctionType.Exp)
            srch = [pp.tile([128, 4 * D], F32, tag=f"srch{h}", name=f"srch{h}")
                    for h in range(2)]
            for h in range(2):
                nc.sync.dma_start(
                    srch[h][:].rearrange("p (a d) -> p a d", a=4),
                    src_d.ap()[512 * h:512 * (h + 1), :].rearrange(
                        "(a p) d -> p a d", p=128))

            def src_slice(i):
                return srch[i // 4][:, D * (i % 4):D * (i % 4 + 1)]
            wpk = pp.tile([128, _NWC], BF16, tag="wpk", name="wpk")
            nc.sync.dma_start(wpk[:], wpack_d.ap())
            bvx = pp.tile([128, 2 * D], F32, tag="bvx", name="bvx")
            nc.sync.dma_start(bvx[:], bvx_d.ap())

            def wv(nm):
                lo, hi = _WCOLS[nm]
                return wpk[:, lo:hi]

            identb = wv("identb")
            wqkT = [wv("wqkT0"), wv("wqkT1")]
            wvxT = [wv("wvxT0"), wv("wvxT1")]
            woT = [wv("woT0"), wv("woT1")]
            w1T = [wv("w1T0"), wv("w1T1")]
            w2T = [wv(f"w2T{k}") for k in range(8)]

            if has_bqk:
                bqk = pp.tile([128, 4], F32, tag="bqk", name="bqk")
                nc.sync.dma_start(bqk[:], bqk_d.ap())
            if aff1:
                g1b = pp.tile([128, D], F32, tag="g1b", name="g1b")
                be1b = pp.tile([128, D], F32, tag="be1b", name="be1b")
                nc.sync.dma_start(g1b[:], g1b_d.ap())
                nc.sync.dma_start(be1b[:], be1b_d.ap())
            if aff2:
                g2b = pp.tile([128, D], F32, tag="g2b", name="g2b")
                be2b = pp.tile([128, D], F32, tag="be2b", name="be2b")
                nc.sync.dma_start(g2b[:], g2b_d.ap())
                nc.sync.dma_start(be2b[:], be2b_d.ap())
            if has_bo:
                bob = pp.tile([128, D], F32, tag="bob", name="bob")
                nc.sync.dma_start(bob[:], bob_d.ap())
            if has_b1:
                b1c = pp.tile([128, DFF // 128], F32, tag="b1c", name="b1c")
                nc.sync.dma_start(b1c[:], b1c_d.ap())
            if has_b2:
                b2c = pp.tile([128, D // 128], F32, tag="b2c", name="b2c")
                nc.sync.dma_start(b2c[:], b2c_d.ap())

            # persistent activations
            xn = [pp.tile([128, D], F32, tag=f"xn{i}", name=f"xn{i}") for i in range(NT)]
            x_res = xn
            if aff1:
                x_res = [pp.tile([128, D], F32, tag=f"xr{i}", name=f"xr{i}") for i in range(NT)]
            xnT = [pp.tile([128, S], BF16, tag=f"xnT{k}", name=f"xnT{k}") for k in range(2)]
            qT = [pp.tile([128, S], BF16, tag=f"qT{k}", name=f"qT{k}") for k in range(2)]
            kT = [pp.tile([128, S], BF16, tag=f"kT{k}", name=f"kT{k}") for k in range(2)]
            vx = [pp.tile([128, 2 * D], BF16, tag=f"vx{i}", name=f"vx{i}") for i in range(NT)]
            ctxT = [pp.tile([128, S], BF16, tag=f"ctxT{k}", name=f"ctxT{k}") for k in range(2)]
            yn = [pp.tile([128, D], F32, tag=f"yn{i}", name=f"yn{i}") for i in range(NT)]
            y_res = yn
            if aff2:
                y_res = [pp.tile([128, D], F32, tag=f"yr{i}", name=f"yr{i}") for i in range(NT)]
            ynT = [pp.tile([128, S], BF16, tag=f"ynT{k}", name=f"ynT{k}") for k in range(2)]
            f1T = [pp.tile([128, S], BF16, tag=f"f1T{m}", name=f"f1T{m}") for m in range(8)]
            f2T = [pp.tile([128, S], BF16, tag=f"f2T{m}", name=f"f2T{m}") for m in range(2)]

            # ================= Phase 1: LN1 -> xnT, qT/kT, vx =================
            with tc.tile_pool(name="work1", bufs=4) as wk, \
                 tc.tile_pool(name="ps1", bufs=2, space="PSUM") as ps1:
                xbf = [pp.tile([128, D], BF16, tag=f"xbf{i}", name=f"xbf{i}") for i in range(NT)]
                ab1 = pp.tile([128, 2 * NT], F32, tag="ab1", name="ab1")
                rs1 = pp.tile([128, NT], F32, tag="rs1", name="rs1")
                lnv1 = pp.tile([128, NT], F32, tag="lnv1", name="lnv1")
                for half in range(2):
                    tiles = range(4 * half, 4 * half + 4)
                    for i in tiles:
                        stats = wk.tile([128, 6], F32, tag="lnstats", name="lnstats")
                        nc.vector.bn_stats(stats[:], src_slice(i))
                        nc.vector.bn_aggr(ab1[:, 2 * i:2 * i + 2], stats[:])
                    # rsqrt(var+eps) = exp(-0.5*ln(var+eps)), batched per half
                    c0 = 4 * half
                    ab1v = ab1[:, 8 * half:8 * half + 8].rearrange(
                        "p (i two) -> p i two", two=2)
                    lnv1v = lnv1[:, c0:c0 + 4].rearrange("p (i o) -> p i o", o=1)
                    nc.scalar.activation(lnv1v, ab1v[:, :, 1:2], LN, bias=eps_t[:, 0:1])
                    nc.scalar.activation(rs1[:, c0:c0 + 4], lnv1[:, c0:c0 + 4],
                                         EXP, scale=-0.5)
                    for i in tiles:
                        nc.vector.tensor_scalar(
                            xn[i][:], src_slice(i),
                            ab1[:, 2 * i:2 * i + 1], rs1[:, i:i + 1],
                            mybir.AluOpType.subtract, mybir.AluOpType.mult)
                        nc.gpsimd.tensor_copy(xbf[i][:], xn[i][:])
                        if aff1:
                            tmp = wk.tile([128, D], F32, tag="afftmp", name="afftmp")
                            nc.vector.tensor_tensor(tmp[:], xn[i][:], g1b[:],
                                                    mybir.AluOpType.mult)
                            nc.vector.tensor_tensor(x_res[i][:], tmp[:], be1b[:],
                                                    mybir.AluOpType.add)
                        # transpose s-tile into xnT columns (both d-blocks)
                        tp = ps1.tile([128, 256], BF16, tag="tp", name="tp")
                        for j in range(2):
                            nc.tensor.transpose(
                                tp[:, 128 * j:128 * (j + 1)],
                                xbf[i][:, 128 * j:128 * (j + 1)], identb)
                        nc.scalar.copy(
                            xnT[0][:, 128 * i:128 * (i + 1)], tp[:, 0:128])
                        nc.vector.tensor_copy(
                            xnT[1][:, 128 * i:128 * (i + 1)], tp[:, 128:256])
                        # v_ext for this tile
                        pv = ps1.tile([128, 512], F32, tag="pv", name="pv")
                        for k in range(2):
                            nc.tensor.matmul(
                                pv[:],
                                xnT[k][:, 128 * i:128 * (i + 1)],
                                wvxT[k],
                                start=(k == 0), stop=(k == 1))
                        nc.vector.tensor_tensor(vx[i][:], pv[:], bvx[:],
                                                mybir.AluOpType.add)
                    # qkT for this s-half
                    for m in range(4):  # 0,1 = q tiles; 2,3 = k tiles
                        dstT = qT[m] if m < 2 else kT[m - 2]
                        pq = ps1.tile([128, 512], F32, tag="pqk", name="pqk")
                        for k in range(2):
                            nc.tensor.matmul(
                                pq[:],
                                wqkT[k][:, 128 * m:128 * (m + 1)],
                                xnT[k][:, 512 * half:512 * (half + 1)],
                                start=(k == 0), stop=(k == 1))
                        if has_bqk:
                            nc.vector.tensor_scalar_add(
                                dstT[:, 512 * half:512 * (half + 1)], pq[:],
                                bqk[:, m:m + 1])
                        else:
                            nc.vector.tensor_copy(
                                dstT[:, 512 * half:512 * (half + 1)], pq[:])

            # ================= Phase 2: attention main loop =================
            # group g = (p, t): head pair (2p, 2p+1), t-tile t.
            # Software pipeline: per iteration issue sc(g), exp(g), mult(g),
            # then ctx(g-1) so the PE always has score matmuls to chew on
            # while ACT/DVE process the previous group.
            with tc.tile_pool(name="battn", bufs=1) as bp, \
                 tc.tile_pool(name="ps2", bufs=1, space="PSUM") as ps2:
                groups = [(p, t) for p in range(4) for t in range(NT)]
                PF = 4  # bias-tile prefetch depth

                bt_tiles = {}

                def fetch_bt(gi):
                    if gi >= len(groups):
                        return
                    bt = bp.tile([128, 2048], BF16, tag="bt", name="bt", bufs=PF + 2)
                    nc.sync.dma_start(bt[:], ebg_d.ap()[gi])
                    bt_tiles[gi] = bt

                for gi in range(PF):
                    fetch_bt(gi)

                sc_tiles = {}
                pt_tiles = {}
                ctx_tiles = {}

                def issue_sc(gi):
                    p, t = groups[gi]
                    h0, h1 = 2 * p, 2 * p + 1
                    b0, b1 = 32 * (h0 % 4), 32 * (h1 % 4)
                    kt, qt = kT[p // 2], qT[p // 2]
                    sc0 = ps2.tile([128, S], F32, tag="sc", name="sc", bufs=3)
                    sc1 = ps2.tile([128, S], F32, tag="sc", name="sc", bufs=3)
                    # interleave bands so adjacent MMs run in different row groups
                    for half in range(2):
                        nc.tensor.matmul(
                            sc0[:, 512 * half:512 * (half + 1)],
                            kt[b0:b0 + 32, 128 * t:128 * (t + 1)],
                            qt[b0:b0 + 32, 512 * half:512 * (half + 1)],
                            start=True, stop=True, tile_position=(b0, 0))
                        nc.tensor.matmul(
                            sc1[:, 512 * half:512 * (half + 1)],
                            kt[b1:b1 + 32, 128 * t:128 * (t + 1)],
                            qt[b1:b1 + 32, 512 * half:512 * (half + 1)],
                            start=True, stop=True, tile_position=(b1, 0))
                    sc_tiles[gi] = (sc0, sc1)

                def issue_exp_mult(gi):
                    sc0, sc1 = sc_tiles.pop(gi)
                    eq = bp.tile([128, 2048], BF16, tag="eq", name="eq", bufs=3)
                    nc.scalar.activation(eq[:, 0:1024], sc0[:], EXP)
                    nc.scalar.activation(eq[:, 1024:2048], sc1[:], EXP)
                    pt = bp.tile([128, 2048], BF16, tag="pt", name="pt", bufs=3)
                    nc.vector.tensor_tensor(pt[:], eq[:], bt_tiles.pop(gi)[:],
                                            mybir.AluOpType.mult)
                    pt_tiles[gi] = pt

                def issue_ctx(gi):
                    p, t = groups[gi]
                    h0, h1 = 2 * p, 2 * p + 1
                    pt = pt_tiles.pop(gi)
                    if t == 0:
                        ctx_tiles[p] = ps2.tile([128, S], F32, tag="ctx",
                                                name="ctx", bufs=1)
                    ctx = ctx_tiles[p]
                    for half in range(2):
                        nc.tensor.matmul(
                            ctx[0:64, 512 * half:512 * (half + 1)],
                            vx[t][:, 64 * h0:64 * (h0 + 1)],
                            pt[:, 512 * half:512 * (half + 1)],
                            start=(t == 0), stop=(t == NT - 1),
                            tile_position=(0, 0), skip_group_check=True)
                        nc.tensor.matmul(
                            ctx[64:128, 512 * half:512 * (half + 1)],
                            vx[t][:, 64 * h1:64 * (h1 + 1)],
                            pt[:, 1024 + 512 * half:1536 + 512 * half],
                            start=(t == 0), stop=(t == NT - 1),
                            tile_position=(0, 64), skip_group_check=True)

                def issue_evac(p):
                    # ctx rows per head: [0:32] ctx, [32:64] denominator
                    # (pre-broadcast via 32 ones-cols in vx); h1 at +64.
                    # 1/den = exp(-ln(den)) on ACT (shares the exp table set).
                    ctx = ctx_tiles.pop(p)
                    for hh in (2 * p, 2 * p + 1):
                        crow = 64 * (hh % 2)
                        band = 32 * (hh % 4)
                        lnd = bp.tile([32, S], F32, tag="lnd", name="lnd", bufs=2)
                        nc.scalar.activation(lnd[:], ctx[crow + 32:crow + 64, :], LN)
                        rr = bp.tile([32, S], F32, tag="rrec", name="rrec", bufs=2)
                        nc.scalar.activation(rr[:], lnd[:], EXP, scale=-1.0)
                        nc.vector.tensor_tensor(
                            ctxT[hh // 4][band:band + 32, :],
                            ctx[crow:crow + 32, :], rr[:],
                            mybir.AluOpType.mult)
                        if _DBG and hh == 0:
                            nc.sync.dma_start(dbg_rr.ap(), rr[:])
                            dd = bp.tile([32, S], F32, tag="dbgden", name="dbgden")
                            nc.vector.tensor_copy(dd[:], ctx[crow + 32:crow + 64, :])
                            nc.sync.dma_start(dbg_den.ap(), dd[:])

                for gi in range(len(groups)):
                    fetch_bt(gi + PF)
                    issue_sc(gi)
                    issue_exp_mult(gi)
                    if gi > 0:
                        issue_ctx(gi - 1)
                        if groups[gi - 1][1] == NT - 1:
                            issue_evac(groups[gi - 1][0])
                issue_ctx(len(groups) - 1)
                issue_evac(3)
                if _DBG:
                    nc.sync.dma_start(dbg_rs1.ap(), rs1[:])
                    dct = bp.tile([128, S], F32, tag="dbgct", name="dbgct")
                    nc.vector.tensor_copy(dct[:], ctxT[0][:])
                    nc.sync.dma_start(dbg_ctxT.ap(), dct[:])

            # ==== Phases 3+4 interleaved by s-half: out-proj + LN2 for a
            # half, then that half's FFN, so DVE (LN2) and PE (FFN) overlap.
            with tc.tile_pool(name="work3", bufs=4) as wk3, \
                 tc.tile_pool(name="ps3", bufs=2, space="PSUM") as ps3:
                ybf = [pp.tile([128, D], BF16, tag=f"ybf{i}", name=f"ybf{i}") for i in range(NT)]
                ab2 = pp.tile([128, 2 * NT], F32, tag="ab2", name="ab2")
                rs2 = pp.tile([128, NT], F32, tag="rs2", name="rs2")
                lnv2 = pp.tile([128, NT], F32, tag="lnv2", name="lnv2")
                identf = pp.tile([128, 128], F32, tag="identf", name="identf")
                nc.vector.tensor_copy(identf[:], identb)
                pas = {}
                for half in range(2):
                    tiles = range(4 * half, 4 * half + 4)
                    pa4 = ps3.tile([128, 4 * D], F32, tag="pa4", name="pa4",
                                   bufs=1)
                    pa_of = lambda i: pa4[:, D * (i % 4):D * (i % 4 + 1)]
                    for i in tiles:
                        pa = pa_of(i)
                        for k in range(2):
                            nc.tensor.matmul(
                                pa,
                                ctxT[k][:, 128 * i:128 * (i + 1)],
                                woT[k],
                                start=(k == 0), stop=False)
                        # residual add on the PE: pa += I @ x_res[i]
                        nc.tensor.matmul(
                            pa, identf[:], x_res[i][:],
                            start=False, stop=True)
                        if has_bo:
                            pab = wk3.tile([128, D], F32, tag="pab", name="pab",
                                           bufs=4)
                            nc.vector.tensor_tensor(pab[:], pa, bob[:],
                                                    mybir.AluOpType.add)
                            pas[i] = pab
                        stats = wk3.tile([128, 6], F32, tag="lnstats", name="lnstats")
                        nc.vector.bn_stats(stats[:], pas[i][:] if has_bo else pa)
                        nc.vector.bn_aggr(ab2[:, 2 * i:2 * i + 2], stats[:])
                    c0 = 4 * half
                    ab2v = ab2[:, 8 * half:8 * half + 8].rearrange(
                        "p (i two) -> p i two", two=2)
                    lnv2v = lnv2[:, c0:c0 + 4].rearrange("p (i o) -> p i o", o=1)
                    nc.scalar.activation(lnv2v, ab2v[:, :, 1:2], LN, bias=eps_t[:, 0:1])
                    nc.scalar.activation(rs2[:, c0:c0 + 4], lnv2[:, c0:c0 + 4],
                                         EXP, scale=-0.5)
                    for i in tiles:
                        src_ap = pas.pop(i)[:] if has_bo else pa_of(i)
                        nc.vector.tensor_scalar(
                            yn[i][:], src_ap, ab2[:, 2 * i:2 * i + 1],
                            rs2[:, i:i + 1],
                            mybir.AluOpType.subtract, mybir.AluOpType.mult)
                        nc.gpsimd.tensor_copy(ybf[i][:], yn[i][:])
                        if aff2:
                            tmp = wk3.tile([128, D], F32, tag="afftmp2", name="afftmp2")
                            nc.vector.tensor_tensor(tmp[:], yn[i][:], g2b[:],
                                                    mybir.AluOpType.mult)
                            nc.vector.tensor_tensor(y_res[i][:], tmp[:], be2b[:],
                                                    mybir.AluOpType.add)
                        tp = ps3.tile([128, 256], BF16, tag="tp3", name="tp3")
                        for j in range(2):
                            nc.tensor.transpose(
                                tp[:, 128 * j:128 * (j + 1)],
                                ybf[i][:, 128 * j:128 * (j + 1)], identb)
                        for j in range(2):
                            nc.vector.tensor_copy(
                                ynT[j][:, 128 * i:128 * (i + 1)],
                                tp[:, 128 * j:128 * (j + 1)])
                    # ---- FFN for this half ----
                    for m in range(8):
                        pf = ps3.tile([128, 512], F32, tag="pf1", name="pf1", bufs=2)
                        for k in range(2):
                            nc.tensor.matmul(
                                pf[:],
                                w1T[k][:, 128 * m:128 * (m + 1)],
                                ynT[k][:, 512 * half:512 * (half + 1)],
                                start=(k == 0), stop=(k == 1))
                        bias_arg = b1c[:, m:m + 1] if has_b1 else 0.0
                        nc.scalar.activation(
                            f1T[m][:, 512 * half:512 * (half + 1)], pf[:],
                            RELU, bias=bias_arg)
                    for m in range(2):
                        pf2 = ps3.tile([128, 512], F32, tag="pf2", name="pf2", bufs=2)
                        for k in range(8):
                            nc.tensor.matmul(
                                pf2[:],
                                w2T[k][:, 128 * m:128 * (m + 1)],
                                f1T[k][:, 512 * half:512 * (half + 1)],
                                start=(k == 0), stop=(k == 7))
                        if has_b2:
                            nc.vector.tensor_scalar_add(
                                f2T[m][:, 512 * half:512 * (half + 1)], pf2[:],
                                b2c[:, m:m + 1])
                        else:
                            nc.vector.tensor_copy(
                                f2T[m][:, 512 * half:512 * (half + 1)], pf2[:])
                    # transpose f2T back + final residual + store, per tile
                    for i in tiles:
                        tpn = ps3.tile([128, D], BF16, tag="tp3", name="tpn")
                        for j in range(2):
                            nc.tensor.transpose(
                                tpn[:, 128 * j:128 * (j + 1)],
                                f2T[j][:, 128 * i:128 * (i + 1)],
                                identb)
                        ot = wk3.tile([128, D], F32, tag="ot", name="ot")
                        nc.vector.tensor_tensor(ot[:], tpn[:], y_res[i][:],
                                                mybir.AluOpType.add)
                        nc.sync.dma_start(out_d.ap()[128 * i:128 * (i + 1), :], ot[:])

    nc.compile()
    return nc


def _prep_host(src, bias, in_proj_w, in_proj_b, out_w, out_b,
               w1, b1, w2, b2, g1, be1, g2, be2):
    f = np.float32
    g1 = np.asarray(g1, f); be1 = np.asarray(be1, f)
    g2 = np.asarray(g2, f); be2 = np.asarray(be2, f)
    in_proj_w = np.asarray(in_proj_w, f); in_proj_b = np.asarray(in_proj_b, f)
    out_w = np.asarray(out_w, f); out_b = np.asarray(out_b, f)
    w1 = np.asarray(w1, f); b1 = np.asarray(b1, f)
    w2 = np.asarray(w2, f); b2 = np.asarray(b2, f)

    winG = in_proj_w * g1[None, :]
    binG = in_proj_w @ be1 + in_proj_b
    scale = HD ** -0.5
    winG[0:D] *= scale
    binG[0:D] *= scale
    wqkT = np.ascontiguousarray(winG[0:2 * D].T).astype(bf16)      # [D, 2D]
    bqk = binG[0:2 * D]                                            # [2D]
    wv = winG[2 * D:3 * D]; bv = binG[2 * D:3 * D]
    # v_ext: head h cols 64h..64h+63: [V_h (32) | ones-block via bvx (32)]
    wvxT = np.zeros((D, 2 * D), f)
    bvx = np.zeros((2 * D,), f)
    for h in range(H):
        wvxT[:, 64 * h:64 * h + 32] = wv[32 * h:32 * h + 32].T
        bvx[64 * h:64 * h + 32] = bv[32 * h:32 * h + 32]
        bvx[64 * h + 32:64 * h + 64] = 1.0
    w1G = w1 * g2[None, :]
    b1p = w1 @ be2 + b1

    flags = (
        bool(np.any(g1 != 1.0) or np.any(be1 != 0.0)),
        bool(np.any(g2 != 1.0) or np.any(be2 != 0.0)),
        bool(np.any(bqk != 0.0)),
        bool(np.any(out_b != 0.0)),
        bool(np.any(b1p != 0.0)),
        bool(np.any(b2 != 0.0)),
    )
    aff1, aff2, has_bqk, has_bo, has_b1, has_b2 = flags

    # packed bf16 weights tile [128, _NWC]
    wpack = np.zeros((128, _NWC), bf16)

    def put(nm, arr):
        lo, hi = _WCOLS[nm]
        wpack[:, lo:hi] = arr.astype(bf16)

    put("identb", np.eye(128, dtype=f))
    w1Gt = np.ascontiguousarray(w1G.T)
    w2t = np.ascontiguousarray(w2.T)
    owt = np.ascontiguousarray(out_w.T)
    for k in range(2):
        put(f"wqkT{k}", wqkT[128 * k:128 * (k + 1), :])
        put(f"wvxT{k}", wvxT[128 * k:128 * (k + 1), :].astype(bf16))
        put(f"woT{k}", owt[128 * k:128 * (k + 1), :])
        put(f"w1T{k}", w1Gt[128 * k:128 * (k + 1), :])
    for k in range(8):
        put(f"w2T{k}", w2t[128 * k:128 * (k + 1), :])

    common = {
        "wpack": wpack,
        "bvx": np.broadcast_to(bvx, (128, 2 * D)).copy(),
    }
    if has_bqk:
        common["bqk"] = np.ascontiguousarray(bqk.reshape(4, 128).T)
    if aff1:
        common["g1b"] = np.broadcast_to(g1, (128, D)).copy()
        common["be1b"] = np.broadcast_to(be1, (128, D)).copy()
    if aff2:
        common["g2b"] = np.broadcast_to(g2, (128, D)).copy()
        common["be2b"] = np.broadcast_to(be2, (128, D)).copy()
    if has_bo:
        common["bob"] = np.broadcast_to(out_b, (128, D)).copy()
    if has_b1:
        common["b1c"] = np.ascontiguousarray(b1p.reshape(DFF // 128, 128).T)
    if has_b2:
        common["b2c"] = np.ascontiguousarray(b2.reshape(D // 128, 128).T)

    src = np.asarray(src, f)
    bias = np.asarray(bias, f)
    # host: exp(bias) transposed -> bf16, regrouped per (head-pair p, t-tile):
    # ebg[8p+t] = [128, 2048] = [expbT[2p][t-tile] | expbT[2p+1][t-tile]]
    expbT = np.exp(bias.transpose(0, 1, 3, 2)).astype(bf16)  # [B, H, S(t), S(s)]
    ebg_all = expbT.reshape(B, 4, 2, NT, 128, S).transpose(0, 1, 3, 4, 2, 5)
    # [B, p, t, 128, h-in-pair, S] -> [B, 32, 128, 2048]
    ebg_all = np.ascontiguousarray(ebg_all.reshape(B, 32, 128, 2048))
    in_maps = []
    for b in range(N_CORES):
        m = dict(common)
        m["src"] = np.ascontiguousarray(src[:, b, :])
        m["ebg"] = ebg_all[b]
        in_maps.append(m)
    return flags, in_maps


def kernel(**inputs):
    _install_axon_hooks_shim()
    flags, in_maps = _prep_host(
        inputs["src"], inputs["bias"], inputs["in_proj_w"], inputs["in_proj_b"],
        inputs["out_w"], inputs["out_b"], inputs["w1"], inputs["b1"],
        inputs["w2"], inputs["b2"], inputs["g1"], inputs["be1"],
        inputs["g2"], inputs["be2"])
    if flags not in _CACHE:
        _CACHE[flags] = _build(flags)
    nc = _CACHE[flags]
    res = run_bass_kernel_spmd(nc, in_maps, core_ids=list(range(N_CORES)))
    out = np.empty((S, B, D), np.float32)
    for b in range(N_CORES):
        out[:, b, :] = res.results[b]["out"]
    return out


# revision 31
# speedup vs baseline: 1.1256x; 1.1256x over previous
"""Trainium2 Bass kernel for a biased transformer encoder layer.

Full (unsharded) inputs -> full output. Internally shards data-parallel over
batch B across 8 NeuronCores (one batch element per core). The bias tensor
(256MB) dominates memory traffic; it is host-exponentiated/transposed to bf16
so attention works in "transposed scores" layout and softmax is
exp(qk)*exp(bias) with a matmul-computed (pre-broadcast) denominator.

v2: software-pipelined attention (PE issues next group's score matmuls before
the previous group's ctx matmuls so it never idles on the exp->mult chain),
all bias multiplies on DVE as single [128,2048] bf16 tensor_tensor ops,
layernorm rsqrt via batched ln/exp on ACT (single activation table set for
the whole kernel), packed single-DMA weight/src loads.
"""

import numpy as np
import ml_dtypes

import concourse.mybir as mybir
import concourse.tile as tile
from concourse import bacc
from concourse.bass_utils import run_bass_kernel_spmd

# ---- problem constants (hardcoded per contract) ----
S = 1024
B = 8
D = 256
H = 8
HD = D // H          # 32
DFF = 1024
EPS = 1e-5
N_CORES = 8
NT = S // 128        # 8 s-tiles / t-tiles

F32 = mybir.dt.float32
BF16 = mybir.dt.bfloat16
bf16 = ml_dtypes.bfloat16

_CACHE = {}

# packed bf16 weight column layout
_WCOLS = {}
_off = 0
for _name, _w in [("identb", 128), ("wqkT0", 512), ("wqkT1", 512),
                  ("wvxT0", 512), ("wvxT1", 512), ("woT0", 256), ("woT1", 256),
                  ("w1T0", 1024), ("w1T1", 1024)] + [(f"w2T{k}", 256) for k in range(8)]:
    _WCOLS[_name] = (_off, _off + _w)
    _off += _w
_NWC = _off  # 6784


def _install_axon_hooks_shim():
    """Make `trace=True` degrade gracefully if antenv.axon_hooks is missing."""
    import sys, types
    try:
        import antenv  # noqa
    except ImportError:
        return
    if "antenv.axon_hooks" in sys.modules:
        return
    try:
        import antenv.axon_hooks  # noqa
    except ImportError:
        import antenv
        mod = types.ModuleType("antenv.axon_hooks")
        _hook = [None]
        mod.set_axon_ntff_profile_hook = lambda h: _hook.__setitem__(0, h)
        mod.get_axon_ntff_profile_hook = lambda: _hook[0]
        sys.modules["antenv.axon_hooks"] = mod
        antenv.axon_hooks = mod


def _patched_act_tables(orig_fn):
    """Return a get_activation_tables wrapper that strips Exp/Ln from every
    set except natural_log_exp_and_others, so the table-load pass resolves
    both functions to the one set that holds them jointly (a single
    ACT_TABLE_LOAD for the whole kernel instead of thrashing between the
    exp-only and ln-only sets)."""
    def patched(arch):
        tabs = {k: set(v) for k, v in orig_fn(arch).items()}
        combined = "natural_log_exp_and_others"
        if combined in tabs:
            EXP = mybir.ActivationFunctionType.Exp
            LNF = mybir.ActivationFunctionType.Ln
            if EXP in tabs[combined] and LNF in tabs[combined]:
                for name, fns in tabs.items():
                    if name != combined:
                        fns.discard(EXP)
                        fns.discard(LNF)
        return tabs
    return patched


def _build(flags):
    """Build the Bass program (shared by all 8 cores, SPMD)."""
    (aff1, aff2, has_bqk, has_bo, has_b1, has_b2, has_bv) = flags
    orig_tables = bacc.get_activation_tables
    bacc.get_activation_tables = _patched_act_tables(orig_tables)
    try:
        return _build_inner(flags)
    finally:
        bacc.get_activation_tables = orig_tables


def _build_inner(flags):
    (aff1, aff2, has_bqk, has_bo, has_b1, has_b2, has_bv) = flags
    nc = bacc.Bacc("TRN2", debug=False, num_devices=N_CORES, enable_asserts=True)

    # ---- DRAM tensors (per-core inputs) ----
    src_d = nc.dram_tensor("src", [S, D], F32, kind="ExternalInput")
    wpack_d = nc.dram_tensor("wpack", [128, _NWC], BF16, kind="ExternalInput")
    if has_bv:
        bvx_d = nc.dram_tensor("bvx", [128, 2 * D], F32, kind="ExternalInput")
    ebg_d = nc.dram_tensor("ebg", [32, 128, 2048], BF16, kind="ExternalInput")
    if has_bqk:
        bqk_d = nc.dram_tensor("bqk", [128, 4], F32, kind="ExternalInput")
    if aff1:
        g1b_d = nc.dram_tensor("g1b", [128, D], F32, kind="ExternalInput")
        be1b_d = nc.dram_tensor("be1b", [128, D], F32, kind="ExternalInput")
    if aff2:
        g2b_d = nc.dram_tensor("g2b", [128, D], F32, kind="ExternalInput")
        be2b_d = nc.dram_tensor("be2b", [128, D], F32, kind="ExternalInput")
    if has_bo:
        bob_d = nc.dram_tensor("bob", [128, D], F32, kind="ExternalInput")
    if has_b1:
        b1c_d = nc.dram_tensor("b1c", [128, DFF // 128], F32, kind="ExternalInput")
    if has_b2:
        b2c_d = nc.dram_tensor("b2c", [128, D // 128], F32, kind="ExternalInput")
    out_d = nc.dram_tensor("out", [S, D], F32, kind="ExternalOutput")
    import os as _os
    _DBG = bool(_os.environ.get("KERNEL_DEBUG"))
    if _DBG:
        dbg_rs1 = nc.dram_tensor("dbg_rs1", [128, NT], F32, kind="ExternalOutput")
        dbg_rr = nc.dram_tensor("dbg_rr", [32, S], F32, kind="ExternalOutput")
        dbg_ctxT = nc.dram_tensor("dbg_ctxT", [128, S], F32, kind="ExternalOutput")
        dbg_den = nc.dram_tensor("dbg_den", [32, S], F32, kind="ExternalOutput")

    LN = mybir.ActivationFunctionType.Ln
    EXP = mybir.ActivationFunctionType.Exp
    RELU = mybir.ActivationFunctionType.Relu

    with tile.TileContext(nc, trace_sim=True) as tc:
        with tc.tile_pool(name="persist", bufs=1) as pp:
            # ---- packed loads ----
            eps_t = pp.tile([128, 1], F32, tag="eps_t", name="eps_t")
            nc.gpsimd.memset(eps_t[:], EPS)
            # prime the ln/exp activation table while DMAs are in flight
            prime = pp.tile([128, 1], F32, tag="prime", name="prime")
            nc.scalar.activation(prime[:], eps_t[:], mybir.ActivationFunctionType.Exp)
            srch = [pp.tile([128, 4 * D], F32, tag=f"srch{h}", name=f"srch{h}")
                    for h in range(2)]
            for h in range(2):
                nc.sync.dma_start(
                    srch[h][:].rearrange("p (a d) -> p a d", a=4),
                    src_d.ap()[512 * h:512 * (h + 1), :].rearrange(
                        "(a p) d -> p a d", p=128))

            def src_slice(i):
                return srch[i // 4][:, D * (i % 4):D * (i % 4 + 1)]
            wpk = pp.tile([128, _NWC], BF16, tag="wpk", name="wpk")
            nc.sync.dma_start(wpk[:], wpack_d.ap())
            if has_bv:
                bvx = pp.tile([128, 2 * D], F32, tag="bvx", name="bvx")
                nc.sync.dma_start(bvx[:], bvx_d.ap())

            def wv(nm):
                lo, hi = _WCOLS[nm]
                return wpk[:, lo:hi]

            identb = wv("identb")
            wqkT = [wv("wqkT0"), wv("wqkT1")]
            wvxT = [wv("wvxT0"), wv("wvxT1")]
            woT = [wv("woT0"), wv("woT1")]
            w1T = [wv("w1T0"), wv("w1T1")]
            w2T = [wv(f"w2T{k}") for k in range(8)]

            if has_bqk:
                bqk = pp.tile([128, 4], F32, tag="bqk", name="bqk")
                nc.sync.dma_start(bqk[:], bqk_d.ap())
            if aff1:
                g1b = pp.tile([128, D], F32, tag="g1b", name="g1b")
                be1b = pp.tile([128, D], F32, tag="be1b", name="be1b")
                nc.sync.dma_start(g1b[:], g1b_d.ap())
                nc.sync.dma_start(be1b[:], be1b_d.ap())
            if aff2:
                g2b = pp.tile([128, D], F32, tag="g2b", name="g2b")
                be2b = pp.tile([128, D], F32, tag="be2b", name="be2b")
                nc.sync.dma_start(g2b[:], g2b_d.ap())
                nc.sync.dma_start(be2b[:], be2b_d.ap())
            if has_bo:
                bob = pp.tile([128, D], F32, tag="bob", name="bob")
                nc.sync.dma_start(bob[:], bob_d.ap())
            if has_b1:
                b1c = pp.tile([128, DFF // 128], F32, tag="b1c", name="b1c")
                nc.sync.dma_start(b1c[:], b1c_d.ap())
            if has_b2:
                b2c = pp.tile([128, D // 128], F32, tag="b2c", name="b2c")
                nc.sync.dma_start(b2c[:], b2c_d.ap())

            # persistent activations (x_res/y_res are bf16 == xbf/ybf unless
            # an affine LN actually needs the separate f32 path)
            if aff1:
                xn = [pp.tile([128, D], F32, tag=f"xn{i}", name=f"xn{i}") for i in range(NT)]
                x_res = [pp.tile([128, D], F32, tag=f"xr{i}", name=f"xr{i}") for i in range(NT)]
            xnT = [pp.tile([128, S], BF16, tag=f"xnT{k}", name=f"xnT{k}") for k in range(2)]
            qT = [pp.tile([128, S], BF16, tag=f"qT{k}", name=f"qT{k}") for k in range(2)]
            kT = [pp.tile([128, S], BF16, tag=f"kT{k}", name=f"kT{k}") for k in range(2)]
            vx = [pp.tile([128, 2 * D], BF16, tag=f"vx{i}", name=f"vx{i}") for i in range(NT)]
            ctxT = [pp.tile([128, S], BF16, tag=f"ctxT{k}", name=f"ctxT{k}") for k in range(2)]
            if aff2:
                yn = [pp.tile([128, D], F32, tag=f"yn{i}", name=f"yn{i}") for i in range(NT)]
                y_res = [pp.tile([128, D], F32, tag=f"yr{i}", name=f"yr{i}") for i in range(NT)]
            ynT = [pp.tile([128, S], BF16, tag=f"ynT{k}", name=f"ynT{k}") for k in range(2)]
            f1T = [pp.tile([128, S], BF16, tag=f"f1T{m}", name=f"f1T{m}") for m in range(8)]
            f2T = [pp.tile([128, S], BF16, tag=f"f2T{m}", name=f"f2T{m}") for m in range(2)]

            # ================= Phase 1: LN1 -> xnT, qT/kT, vx =================
            with tc.tile_pool(name="work1", bufs=4) as wk, \
                 tc.tile_pool(name="ps1", bufs=2, space="PSUM") as ps1:
                xbf = [pp.tile([128, D], BF16, tag=f"xbf{i}", name=f"xbf{i}") for i in range(NT)]
                if not aff1:
                    x_res = xbf
                ab1 = pp.tile([128, 2 * NT], F32, tag="ab1", name="ab1")
                rs1 = pp.tile([128, NT], F32, tag="rs1", name="rs1")
                lnv1 = pp.tile([128, NT], F32, tag="lnv1", name="lnv1")
                if not has_bv:
                    # ones-blocks of v_ext are constant: set once, then only
                    # the V columns get (strided) copied from PSUM per tile
                    for i in range(NT):
                        for h in range(H):
                            nc.gpsimd.memset(vx[i][:, 64 * h + 32:64 * h + 64], 1.0)
                for half in range(2):
                    tiles = range(4 * half, 4 * half + 4)
                    for i in tiles:
                        stats = wk.tile([128, 6], F32, tag="lnstats", name="lnstats")
                        nc.vector.bn_stats(stats[:], src_slice(i))
                        nc.vector.bn_aggr(ab1[:, 2 * i:2 * i + 2], stats[:])
                    # rsqrt(var+eps) = exp(-0.5*ln(var+eps)), batched per half
                    c0 = 4 * half
                    ab1v = ab1[:, 8 * half:8 * half + 8].rearrange(
                        "p (i two) -> p i two", two=2)
                    lnv1v = lnv1[:, c0:c0 + 4].rearrange("p (i o) -> p i o", o=1)
                    nc.scalar.activation(lnv1v, ab1v[:, :, 1:2], LN, bias=eps_t[:, 0:1])
                    nc.scalar.activation(rs1[:, c0:c0 + 4], lnv1[:, c0:c0 + 4],
                                         EXP, scale=-0.5)
                    for i in tiles:
                        if aff1:
                            nc.vector.tensor_scalar(
                                xn[i][:], src_slice(i),
                                ab1[:, 2 * i:2 * i + 1], rs1[:, i:i + 1],
                                mybir.AluOpType.subtract, mybir.AluOpType.mult)
                            nc.gpsimd.tensor_copy(xbf[i][:], xn[i][:])
                            tmp = wk.tile([128, D], F32, tag="afftmp", name="afftmp")
                            nc.vector.tensor_tensor(tmp[:], xn[i][:], g1b[:],
                                                    mybir.AluOpType.mult)
                            nc.vector.tensor_tensor(x_res[i][:], tmp[:], be1b[:],
                                                    mybir.AluOpType.add)
                        else:
                            nc.vector.tensor_scalar(
                                xbf[i][:], src_slice(i),
                                ab1[:, 2 * i:2 * i + 1], rs1[:, i:i + 1],
                                mybir.AluOpType.subtract, mybir.AluOpType.mult)
                        # transpose s-tile into xnT columns (both d-blocks)
                        tp = ps1.tile([128, 256], BF16, tag="tp", name="tp")
                        for j in range(2):
                            nc.tensor.transpose(
                                tp[:, 128 * j:128 * (j + 1)],
                                xbf[i][:, 128 * j:128 * (j + 1)], identb)
                        nc.scalar.copy(
                            xnT[0][:, 128 * i:128 * (i + 1)], tp[:, 0:128])
                        nc.vector.tensor_copy(
                            xnT[1][:, 128 * i:128 * (i + 1)], tp[:, 128:256])
                        # v_ext for this tile
                        pv = ps1.tile([128, 512], F32, tag="pv", name="pv")
                        for k in range(2):
                            nc.tensor.matmul(
                                pv[:],
                                xnT[k][:, 128 * i:128 * (i + 1)],
                                wvxT[k],
                                start=(k == 0), stop=(k == 1))
                        if has_bv:
                            nc.vector.tensor_tensor(vx[i][:], pv[:], bvx[:],
                                                    mybir.AluOpType.add)
                        else:
                            vxv = vx[i][:].rearrange("p (h c) -> p h c", c=64)
                            pvv = pv[:].rearrange("p (h c) -> p h c", c=64)
                            nc.vector.tensor_copy(vxv[:, :, 0:32], pvv[:, :, 0:32])
                    # qkT for this s-half (copies on ACT: idle during phase 1)
                    for m in range(4):  # 0,1 = q tiles; 2,3 = k tiles
                        dstT = qT[m] if m < 2 else kT[m - 2]
                        pq = ps1.tile([128, 512], F32, tag="pqk", name="pqk")
                        for k in range(2):
                            nc.tensor.matmul(
                                pq[:],
                                wqkT[k][:, 128 * m:128 * (m + 1)],
                                xnT[k][:, 512 * half:512 * (half + 1)],
                                start=(k == 0), stop=(k == 1))
                        if has_bqk:
                            nc.vector.tensor_scalar_add(
                                dstT[:, 512 * half:512 * (half + 1)], pq[:],
                                bqk[:, m:m + 1])
                        else:
                            nc.scalar.copy(
                                dstT[:, 512 * half:512 * (half + 1)], pq[:])

            # ================= Phase 2: attention main loop =================
            # group g = (p, t): head pair (2p, 2p+1), t-tile t.
            # Software pipeline: per iteration issue sc(g), exp(g), mult(g),
            # then ctx(g-1) so the PE always has score matmuls to chew on
            # while ACT/DVE process the previous group.
            with tc.tile_pool(name="battn", bufs=1) as bp, \
                 tc.tile_pool(name="ps2", bufs=1, space="PSUM") as ps2:
                groups = [(p, t) for p in range(4) for t in range(NT)]
                PF = 4  # bias-tile prefetch depth

                bt_tiles = {}

                def fetch_bt(gi):
                    if gi >= len(groups):
                        return
                    bt = bp.tile([128, 2048], BF16, tag="bt", name="bt", bufs=PF + 2)
                    nc.sync.dma_start(bt[:], ebg_d.ap()[gi])
                    bt_tiles[gi] = bt

                for gi in range(PF):
                    fetch_bt(gi)

                sc_tiles = {}
                pt_tiles = {}
                ctx_tiles = {}

                def issue_sc(gi):
                    p, t = groups[gi]
                    h0, h1 = 2 * p, 2 * p + 1
                    b0, b1 = 32 * (h0 % 4), 32 * (h1 % 4)
                    kt, qt = kT[p // 2], qT[p // 2]
                    sc0 = ps2.tile([128, S], F32, tag="sc", name="sc", bufs=3)
                    sc1 = ps2.tile([128, S], F32, tag="sc", name="sc", bufs=3)
                    # interleave bands so adjacent MMs run in different row groups
                    for half in range(2):
                        nc.tensor.matmul(
                            sc0[:, 512 * half:512 * (half + 1)],
                            kt[b0:b0 + 32, 128 * t:128 * (t + 1)],
                            qt[b0:b0 + 32, 512 * half:512 * (half + 1)],
                            start=True, stop=True, tile_position=(b0, 0))
                        nc.tensor.matmul(
                            sc1[:, 512 * half:512 * (half + 1)],
                            kt[b1:b1 + 32, 128 * t:128 * (t + 1)],
                            qt[b1:b1 + 32, 512 * half:512 * (half + 1)],
                            start=True, stop=True, tile_position=(b1, 0))
                    sc_tiles[gi] = (sc0, sc1)

                def issue_exp_mult(gi):
                    sc0, sc1 = sc_tiles.pop(gi)
                    eq = bp.tile([128, 2048], BF16, tag="eq", name="eq", bufs=3)
                    nc.scalar.activation(eq[:, 0:1024], sc0[:], EXP)
                    nc.scalar.activation(eq[:, 1024:2048], sc1[:], EXP)
                    pt = bp.tile([128, 2048], BF16, tag="pt", name="pt", bufs=3)
                    nc.vector.tensor_tensor(pt[:], eq[:], bt_tiles.pop(gi)[:],
                                            mybir.AluOpType.mult)
                    pt_tiles[gi] = pt

                def issue_ctx(gi):
                    p, t = groups[gi]
                    h0, h1 = 2 * p, 2 * p + 1
                    pt = pt_tiles.pop(gi)
                    if t == 0:
                        ctx_tiles[p] = ps2.tile([128, S], F32, tag="ctx",
                                                name="ctx", bufs=1)
                    ctx = ctx_tiles[p]
                    for half in range(2):
                        nc.tensor.matmul(
                            ctx[0:64, 512 * half:512 * (half + 1)],
                            vx[t][:, 64 * h0:64 * (h0 + 1)],
                            pt[:, 512 * half:512 * (half + 1)],
                            start=(t == 0), stop=(t == NT - 1),
                            tile_position=(0, 0), skip_group_check=True)
                        nc.tensor.matmul(
                            ctx[64:128, 512 * half:512 * (half + 1)],
                            vx[t][:, 64 * h1:64 * (h1 + 1)],
                            pt[:, 1024 + 512 * half:1536 + 512 * half],
                            start=(t == 0), stop=(t == NT - 1),
                            tile_position=(0, 64), skip_group_check=True)

                def issue_evac(p):
                    # ctx rows per head: [0:32] ctx, [32:64] denominator
                    # (pre-broadcast via 32 ones-cols in vx); h1 at +64.
                    # 1/den = exp(-ln(den)) on ACT (shares the exp table set).
                    ctx = ctx_tiles.pop(p)
                    for hh in (2 * p, 2 * p + 1):
                        crow = 64 * (hh % 2)
                        band = 32 * (hh % 4)
                        lnd = bp.tile([32, S], F32, tag="lnd", name="lnd", bufs=2)
                        nc.scalar.activation(lnd[:], ctx[crow + 32:crow + 64, :], LN)
                        rr = bp.tile([32, S], F32, tag="rrec", name="rrec", bufs=2)
                        nc.scalar.activation(rr[:], lnd[:], EXP, scale=-1.0)
                        nc.vector.tensor_tensor(
                            ctxT[hh // 4][band:band + 32, :],
                            ctx[crow:crow + 32, :], rr[:],
                            mybir.AluOpType.mult)
                        if _DBG and hh == 0:
                            nc.sync.dma_start(dbg_rr.ap(), rr[:])
                            dd = bp.tile([32, S], F32, tag="dbgden", name="dbgden")
                            nc.vector.tensor_copy(dd[:], ctx[crow + 32:crow + 64, :])
                            nc.sync.dma_start(dbg_den.ap(), dd[:])

                for gi in range(len(groups)):
                    fetch_bt(gi + PF)
                    issue_sc(gi)
                    issue_exp_mult(gi)
                    if gi > 0:
                        issue_ctx(gi - 1)
                        if groups[gi - 1][1] == NT - 1:
                            issue_evac(groups[gi - 1][0])
                issue_ctx(len(groups) - 1)
                issue_evac(3)
                if _DBG:
                    nc.sync.dma_start(dbg_rs1.ap(), rs1[:])
                    dct = bp.tile([128, S], F32, tag="dbgct", name="dbgct")
                    nc.vector.tensor_copy(dct[:], ctxT[0][:])
                    nc.sync.dma_start(dbg_ctxT.ap(), dct[:])

            # ==== Phases 3+4 interleaved by s-half: out-proj + LN2 for a
            # half, then that half's FFN, so DVE (LN2) and PE (FFN) overlap.
            with tc.tile_pool(name="work3", bufs=4) as wk3, \
                 tc.tile_pool(name="ps3", bufs=2, space="PSUM") as ps3:
                ybf = [pp.tile([128, D], BF16, tag=f"ybf{i}", name=f"ybf{i}") for i in range(NT)]
                if not aff2:
                    y_res = ybf
                ab2 = pp.tile([128, 2 * NT], F32, tag="ab2", name="ab2")
                rs2 = pp.tile([128, NT], F32, tag="rs2", name="rs2")
                lnv2 = pp.tile([128, NT], F32, tag="lnv2", name="lnv2")
                hts = {}
                for half in range(2):
                    tiles = range(4 * half, 4 * half + 4)
                    for i in tiles:
                        pa = ps3.tile([128, D], F32, tag="pattn", name="pattn",
                                      bufs=2)
                        for k in range(2):
                            nc.tensor.matmul(
                                pa[:],
                                ctxT[k][:, 128 * i:128 * (i + 1)],
                                woT[k],
                                start=(k == 0), stop=(k == 1))
                        ht = wk3.tile([128, D], F32, tag="ht", name="ht", bufs=5)
                        nc.vector.tensor_tensor(ht[:], pa[:], x_res[i][:],
                                                mybir.AluOpType.add)
                        if has_bo:
                            ht2 = wk3.tile([128, D], F32, tag="ht2", name="ht2",
                                           bufs=5)
                            nc.vector.tensor_tensor(ht2[:], ht[:], bob[:],
                                                    mybir.AluOpType.add)
                            ht = ht2
                        hts[i] = ht
                        stats = wk3.tile([128, 6], F32, tag="lnstats", name="lnstats")
                        nc.vector.bn_stats(stats[:], ht[:])
                        nc.vector.bn_aggr(ab2[:, 2 * i:2 * i + 2], stats[:])
                    c0 = 4 * half
                    ab2v = ab2[:, 8 * half:8 * half + 8].rearrange(
                        "p (i two) -> p i two", two=2)
                    lnv2v = lnv2[:, c0:c0 + 4].rearrange("p (i o) -> p i o", o=1)
                    nc.scalar.activation(lnv2v, ab2v[:, :, 1:2], LN, bias=eps_t[:, 0:1])
                    nc.scalar.activation(rs2[:, c0:c0 + 4], lnv2[:, c0:c0 + 4],
                                         EXP, scale=-0.5)
                    for i in tiles:
                        ht = hts.pop(i)
                        if aff2:
                            nc.vector.tensor_scalar(
                                yn[i][:], ht[:], ab2[:, 2 * i:2 * i + 1],
                                rs2[:, i:i + 1],
                                mybir.AluOpType.subtract, mybir.AluOpType.mult)
                            nc.gpsimd.tensor_copy(ybf[i][:], yn[i][:])
                            tmp = wk3.tile([128, D], F32, tag="afftmp2", name="afftmp2")
                            nc.vector.tensor_tensor(tmp[:], yn[i][:], g2b[:],
                                                    mybir.AluOpType.mult)
                            nc.vector.tensor_tensor(y_res[i][:], tmp[:], be2b[:],
                                                    mybir.AluOpType.add)
                        else:
                            nc.vector.tensor_scalar(
                                ybf[i][:], ht[:], ab2[:, 2 * i:2 * i + 1],
                                rs2[:, i:i + 1],
                                mybir.AluOpType.subtract, mybir.AluOpType.mult)
                        tp = ps3.tile([128, 256], BF16, tag="tp3", name="tp3")
                        for j in range(2):
                            nc.tensor.transpose(
                                tp[:, 128 * j:128 * (j + 1)],
                                ybf[i][:, 128 * j:128 * (j + 1)], identb)
                        for j in range(2):
                            nc.vector.tensor_copy(
                                ynT[j][:, 128 * i:128 * (i + 1)],
                                tp[:, 128 * j:128 * (j + 1)])
                    # ---- FFN for this half ----
                    for m in range(8):
                        pf = ps3.tile([128, 512], F32, tag="pf1", name="pf1", bufs=2)
                        for k in range(2):
                            nc.tensor.matmul(
                                pf[:],
                                w1T[k][:, 128 * m:128 * (m + 1)],
                                ynT[k][:, 512 * half:512 * (half + 1)],
                                start=(k == 0), stop=(k == 1))
                        bias_arg = b1c[:, m:m + 1] if has_b1 else 0.0
                        nc.scalar.activation(
                            f1T[m][:, 512 * half:512 * (half + 1)], pf[:],
                            RELU, bias=bias_arg)
                    for m in range(2):
                        pf2 = ps3.tile([128, 512], F32, tag="pf2", name="pf2", bufs=2)
                        for k in range(8):
                            nc.tensor.matmul(
                                pf2[:],
                                w2T[k][:, 128 * m:128 * (m + 1)],
                                f1T[k][:, 512 * half:512 * (half + 1)],
                                start=(k == 0), stop=(k == 7))
                        if has_b2:
                            nc.vector.tensor_scalar_add(
                                f2T[m][:, 512 * half:512 * (half + 1)], pf2[:],
                                b2c[:, m:m + 1])
                        else:
                            nc.vector.tensor_copy(
                                f2T[m][:, 512 * half:512 * (half + 1)], pf2[:])
                    # transpose f2T back + final residual + store, per tile
                    for i in tiles:
                        tpn = ps3.tile([128, D], BF16, tag="tp3", name="tpn")
                        for j in range(2):
                            nc.tensor.transpose(
                                tpn[:, 128 * j:128 * (j + 1)],
                                f2T[j][:, 128 * i:128 * (i + 1)],
                                identb)
                        ot = wk3.tile([128, D], F32, tag="ot", name="ot")
                        nc.vector.tensor_tensor(ot[:], tpn[:], y_res[i][:],
                                                mybir.AluOpType.add)
                        nc.sync.dma_start(out_d.ap()[128 * i:128 * (i + 1), :], ot[:])

    nc.compile()
    return nc


def _prep_host(src, bias, in_proj_w, in_proj_b, out_w, out_b,
               w1, b1, w2, b2, g1, be1, g2, be2):
    f = np.float32
    g1 = np.asarray(g1, f); be1 = np.asarray(be1, f)
    g2 = np.asarray(g2, f); be2 = np.asarray(be2, f)
    in_proj_w = np.asarray(in_proj_w, f); in_proj_b = np.asarray(in_proj_b, f)
    out_w = np.asarray(out_w, f); out_b = np.asarray(out_b, f)
    w1 = np.asarray(w1, f); b1 = np.asarray(b1, f)
    w2 = np.asarray(w2, f); b2 = np.asarray(b2, f)

    winG = in_proj_w * g1[None, :]
    binG = in_proj_w @ be1 + in_proj_b
    scale = HD ** -0.5
    winG[0:D] *= scale
    binG[0:D] *= scale
    wqkT = np.ascontiguousarray(winG[0:2 * D].T).astype(bf16)      # [D, 2D]
    bqk = binG[0:2 * D]                                            # [2D]
    wv = winG[2 * D:3 * D]; bv = binG[2 * D:3 * D]
    # v_ext: head h cols 64h..64h+63: [V_h (32) | ones-block via bvx (32)]
    wvxT = np.zeros((D, 2 * D), f)
    bvx = np.zeros((2 * D,), f)
    for h in range(H):
        wvxT[:, 64 * h:64 * h + 32] = wv[32 * h:32 * h + 32].T
        bvx[64 * h:64 * h + 32] = bv[32 * h:32 * h + 32]
        bvx[64 * h + 32:64 * h + 64] = 1.0
    w1G = w1 * g2[None, :]
    b1p = w1 @ be2 + b1

    flags = (
        bool(np.any(g1 != 1.0) or np.any(be1 != 0.0)),
        bool(np.any(g2 != 1.0) or np.any(be2 != 0.0)),
        bool(np.any(bqk != 0.0)),
        bool(np.any(out_b != 0.0)),
        bool(np.any(b1p != 0.0)),
        bool(np.any(b2 != 0.0)),
        bool(np.any(bv != 0.0)),
    )
    aff1, aff2, has_bqk, has_bo, has_b1, has_b2, has_bv = flags

    # packed bf16 weights tile [128, _NWC]
    wpack = np.zeros((128, _NWC), bf16)

    def put(nm, arr):
        lo, hi = _WCOLS[nm]
        wpack[:, lo:hi] = arr.astype(bf16)

    put("identb", np.eye(128, dtype=f))
    w1Gt = np.ascontiguousarray(w1G.T)
    w2t = np.ascontiguousarray(w2.T)
    owt = np.ascontiguousarray(out_w.T)
    for k in range(2):
        put(f"wqkT{k}", wqkT[128 * k:128 * (k + 1), :])
        put(f"wvxT{k}", wvxT[128 * k:128 * (k + 1), :].astype(bf16))
        put(f"woT{k}", owt[128 * k:128 * (k + 1), :])
        put(f"w1T{k}", w1Gt[128 * k:128 * (k + 1), :])
    for k in range(8):
        put(f"w2T{k}", w2t[128 * k:128 * (k + 1), :])

    common = {
        "wpack": wpack,
    }
    if has_bv:
        common["bvx"] = np.broadcast_to(bvx, (128, 2 * D)).copy()
    if has_bqk:
        common["bqk"] = np.ascontiguousarray(bqk.reshape(4, 128).T)
    if aff1:
        common["g1b"] = np.broadcast_to(g1, (128, D)).copy()
        common["be1b"] = np.broadcast_to(be1, (128, D)).copy()
    if aff2:
        common["g2b"] = np.broadcast_to(g2, (128, D)).copy()
        common["be2b"] = np.broadcast_to(be2, (128, D)).copy()
    if has_bo:
        common["bob"] = np.broadcast_to(out_b, (128, D)).copy()
    if has_b1:
        common["b1c"] = np.ascontiguousarray(b1p.reshape(DFF // 128, 128).T)
    if has_b2:
        common["b2c"] = np.ascontiguousarray(b2.reshape(D // 128, 128).T)

    src = np.asarray(src, f)
    bias = np.asarray(bias, f)
    # host: exp(bias) transposed -> bf16, regrouped per (head-pair p, t-tile):
    # ebg[8p+t] = [128, 2048] = [expbT[2p][t-tile] | expbT[2p+1][t-tile]]
    expbT = np.exp(bias.transpose(0, 1, 3, 2)).astype(bf16)  # [B, H, S(t), S(s)]
    ebg_all = expbT.reshape(B, 4, 2, NT, 128, S).transpose(0, 1, 3, 4, 2, 5)
    # [B, p, t, 128, h-in-pair, S] -> [B, 32, 128, 2048]
    ebg_all = np.ascontiguousarray(ebg_all.reshape(B, 32, 128, 2048))
    in_maps = []
    for b in range(N_CORES):
        m = dict(common)
        m["src"] = np.ascontiguousarray(src[:, b, :])
        m["ebg"] = ebg_all[b]
        in_maps.append(m)
    return flags, in_maps


def kernel(**inputs):
    _install_axon_hooks_shim()
    flags, in_maps = _prep_host(
        inputs["src"], inputs["bias"], inputs["in_proj_w"], inputs["in_proj_b"],
        inputs["out_w"], inputs["out_b"], inputs["w1"], inputs["b1"],
        inputs["w2"], inputs["b2"], inputs["g1"], inputs["be1"],
        inputs["g2"], inputs["be2"])
    if flags not in _CACHE:
        _CACHE[flags] = _build(flags)
    nc = _CACHE[flags]
    res = run_bass_kernel_spmd(nc, in_maps, core_ids=list(range(N_CORES)))
    out = np.empty((S, B, D), np.float32)
    for b in range(N_CORES):
        out[:, b, :] = res.results[b]["out"]
    return out
